# revision 42
# baseline (speedup 1.0000x reference)
"""Trainium2 Bass kernel for nn_CausalAttention (GNN message passing).

Math (reference):
    pairs[e] = [img[:, src[e]] ; text[:, tgt[e]]]          # B == H == 128
    a[e]     = sigmoid(w2 . relu(W1 @ pairs[e] + b1) + b2) # per-edge gate
    att_img[b, i] = sum_{e: src[e]=i} a[e] * text[b, tgt[e]]
    att_txt[b, t] = sum_{e: tgt[e]=t} a[e] * img[b, src[e]]

v9 architecture: deduplicated edges + host reduction, fp16/fp8 on-chip.
Core c owns the edges with src in Wc = [128c, 128c+128). It computes
att_img[:, Wc] exactly plus a PARTIAL att_txt[:, :]; the host sums the
8 partials (no collectives). Per-edge work is done once per edge.

Edges are bucketed by w = tgt>>7; each core PERMUTES its buckets by
fill (descending) so a shared capacity profile (max over cores of the
k-th largest fill, in 128-blocks) minimizes padding; the host permutes
txt/ttT8 chunks to match and un-permutes the out_part columns.

Host ships index-derived one-hot tables in fp8 (they hold only 0/1;
fp8 halves DMA, and the PE accepts f16 x f8 matmuls):
  ohkt [loc, e] / ohlt [lo, e]  key-major (phase A gathers)
  ohk  [e, loc] / ohlo [e, lo]  edge-major (phase B outer products)
Per bucket slot k (cap = 512 or 640 edge slots):
  h   = relu(UwinT.T @ ohkt + V8[k].T @ ohlt + b1)   2-4 mm + ACT
  za  = h_blk.T @ w2 (N=1 mms into mtp), a = sigmoid(za + b2)
  ohka= ohk * a  (ONE broadcast tensor_tensor on DVE)
  M_T[lo, loc] += ohlo_blk.T @ ohka_blk  (PSUM group in mtp)
  M_N = PE-transpose(M_T) (f16 into bitcast mtp region)
  acc_img += ttT8[k].T @ M_T             (long PSUM group)
  part[:, k] = imgwinT.T @ M_N           -> SBUF -> DMA out
PSUM banks: h x2 (4) + mtp x3 (3: mT|part|M_N|za) + acc (1) = 8.
U/V feature transforms built on-chip in fp16, interleaved with the
first buckets so the PE never queues behind un-arrived DMA chunks.
"""

import sys

for _p in ("/opt/trn_rl_repo", "/root/.axon_site/_ro/trn_rl_repo"):
    if _p not in sys.path:
        sys.path.insert(0, _p)

import numpy as np

import concourse.bass as bass
import concourse.tile as tile
from concourse import bacc, mybir

P = 128
DIM = 1024
NCORES = 8
NW = 8            # tgt-hi buckets

F32 = mybir.dt.float32
F16 = mybir.dt.float16
F8 = mybir.dt.float8e4

IS_EQ = mybir.AluOpType.is_equal
MULT = mybir.AluOpType.mult
RELU = mybir.ActivationFunctionType.Relu
SIGMOID = mybir.ActivationFunctionType.Sigmoid


def _build_program(blocks):
    caps = [b * P for b in blocks]           # slot capacities (edge slots)
    off = np.concatenate([[0], np.cumsum(caps)]).astype(int)
    ec = int(off[-1])

    nc = bacc.Bacc(None, target_bir_lowering=False, debug=False)

    txt16_d = nc.dram_tensor("txt16", [P, DIM], F16, kind="ExternalInput")
    ttT8_d = nc.dram_tensor("ttT8", [P, NW * P], F16, kind="ExternalInput")
    # blob: w1i | w1x | iw | iwT | w2h | pad | b1,b2 (f32 as 4 f16 cols)
    blob_d = nc.dram_tensor("blob16", [P, 4 * P + 6], F16, kind="ExternalInput")
    ohkt_d = nc.dram_tensor("ohkt", [P, ec], F8, kind="ExternalInput")
    ohlt_d = nc.dram_tensor("ohlt", [P, ec], F8, kind="ExternalInput")
    ohlo_d = nc.dram_tensor("ohlo", [P, ec], F8, kind="ExternalInput")
    ohk_d = nc.dram_tensor("ohk", [P, ec], F8, kind="ExternalInput")
    out_img = nc.dram_tensor("out_img", [P, P], F32, kind="ExternalOutput")
    out_part = nc.dram_tensor("out_part", [P, DIM], F32, kind="ExternalOutput")

    HW = 640

    with tile.TileContext(nc) as tc:
        with (
            tc.tile_pool(name="const", bufs=1) as cp,
            tc.tile_pool(name="work", bufs=4) as wp,
            tc.tile_pool(name="psh", bufs=2, space="PSUM") as psh,
            tc.tile_pool(name="psm", bufs=3, space="PSUM") as psm,
            tc.tile_pool(name="pso", bufs=1, space="PSUM") as pso,
        ):
            txt16 = cp.tile([P, DIM], F16)
            ttT8 = cp.tile([P, NW, P], F16)
            blob_s = cp.tile([P, 4 * P + 6], F16)
            w1i_s = blob_s[:, 0:P]
            w1x_s = blob_s[:, P : 2 * P]
            iw_s = blob_s[:, 2 * P : 3 * P]
            iwT_s = blob_s[:, 3 * P : 4 * P]
            w2h_s = blob_s[:, 4 * P : 4 * P + 1]
            cst_s = blob_s[:, 4 * P + 2 : 4 * P + 6].bitcast(F32)
            ohkt_s = cp.tile([P, ec], F8)
            ohlt_s = cp.tile([P, ec], F8)
            ohlo_s = cp.tile([P, ec], F8)
            ohk_s = cp.tile([P, ec], F8)
            part_all = cp.tile([P, DIM], F32)
            iota16 = cp.tile([P, P], F16)
            iota_i = cp.tile([P, 1], mybir.dt.int32)
            iota_cf = cp.tile([P, 1], F32)
            ident16 = cp.tile([P, P], F16)
            V8 = cp.tile([P, NW, P], F16)
            UwinT = cp.tile([P, P], F16)
            a_s = cp.tile([P, NW * 5], F32)

            # DMA plan: scalar(Act) queue issues the ohlt/ohk chunks;
            # sync(SP) gets builds' inputs first, then ohkt/ohlo chunks.
            # Chunks split at slot boundaries 2 and 4.
            CH = [0, int(off[2]), int(off[4]), ec]
            for a, b in zip(CH[:-1], CH[1:]):
                nc.scalar.dma_start(ohlt_s[:, a:b], ohlt_d[:, a:b])
                nc.scalar.dma_start(ohk_s[:, a:b], ohk_d[:, a:b])
            TC = [0, 2 * P, 4 * P, DIM]
            nc.sync.dma_start(txt16[:, TC[0] : TC[1]], txt16_d[:, TC[0] : TC[1]])
            nc.sync.dma_start(blob_s[:], blob_d[:])
            nc.sync.dma_start(txt16[:, TC[1] : TC[2]], txt16_d[:, TC[1] : TC[2]])
            nc.sync.dma_start(ohkt_s[:, CH[0] : CH[1]], ohkt_d[:, CH[0] : CH[1]])
            nc.sync.dma_start(ohlo_s[:, CH[0] : CH[1]], ohlo_d[:, CH[0] : CH[1]])
            nc.sync.dma_start(txt16[:, TC[2] : TC[3]], txt16_d[:, TC[2] : TC[3]])
            nc.sync.dma_start(ohkt_s[:, CH[1] : CH[2]], ohkt_d[:, CH[1] : CH[2]])
            nc.sync.dma_start(ohlo_s[:, CH[1] : CH[2]], ohlo_d[:, CH[1] : CH[2]])
            nc.sync.dma_start(ohkt_s[:, CH[2] : CH[3]], ohkt_d[:, CH[2] : CH[3]])
            nc.sync.dma_start(ohlo_s[:, CH[2] : CH[3]], ohlo_d[:, CH[2] : CH[3]])
            nc.sync.dma_start(
                ttT8[:], ttT8_d[:].rearrange("p (w b) -> p w b", w=NW)
            )
            b1c = cst_s[:, 0:1]
            b2c = cst_s[:, 1:2]

            nc.gpsimd.iota(
                iota16[:], pattern=[[1, P]], base=0, channel_multiplier=0,
                allow_small_or_imprecise_dtypes=True,
            )
            nc.gpsimd.iota(iota_i[:], pattern=[[0, 1]], base=0,
                           channel_multiplier=1)
            nc.vector.tensor_copy(iota_cf[:], iota_i[:])
            nc.vector.tensor_scalar(
                out=ident16[:], in0=iota16[:], scalar1=iota_cf[:, 0:1],
                scalar2=None, op0=IS_EQ,
            )

            # warm-up matmuls: keep the PE continuously busy while input
            # tables stream in, so DVFS ramps the clock before real work.
            for wi in range(20):
                wt = psm.tile([P, 384], F32, tag="mtp", name=f"warm{wi}")
                nc.tensor.matmul(
                    wt[:, 0:P], iota16[:], ident16[:],
                    start=True, stop=True, skip_group_check=True,
                )

            def build(lhs, rhs, dst, name):
                bp = psh.tile([P, HW], F32, tag="h", name=name)
                nc.tensor.matmul(bp[:, 0:P], lhs, rhs, start=True, stop=True)
                nc.vector.tensor_copy(dst, bp[:, 0:P])

            def vbuild(k):
                build(txt16[:, k * P : (k + 1) * P], w1x_s, V8[:, k, :],
                      f"v{k}")

            build(iw_s, w1i_s, UwinT[:], "u")
            vbuild(0)
            vbuild(1)

            acc = pso.tile([P, P], F32, tag="acc")
            for k in range(NW):
                cap = caps[k]
                nb = blocks[k]
                e0 = int(off[k])
                # interleave remaining V8 builds with the first buckets
                if k == 0:
                    vbuild(2), vbuild(3)
                elif k == 1:
                    vbuild(4), vbuild(5)
                elif k == 2:
                    vbuild(6), vbuild(7)
                # mtp bank layout (f32 cols): [0:128] M_T accum,
                # [128:256] part chunk, [256:320] M_N (f16 bitcast),
                # [320:325] za.  All groups sequential within the bank.
                mtp = psm.tile([P, 384], F32, tag="mtp")
                # ---- phase A: h = relu(U-term + V-term + b1) ----
                h_ps = psh.tile([P, HW], F32, tag="h")
                for o, n in ((0, 512), (512, cap - 512)):
                    if n <= 0:
                        continue
                    nc.tensor.matmul(
                        h_ps[:, o : o + n], UwinT[:],
                        ohkt_s[:, e0 + o : e0 + o + n],
                        start=True, stop=False,
                    )
                    nc.tensor.matmul(
                        h_ps[:, o : o + n], V8[:, k, :],
                        ohlt_s[:, e0 + o : e0 + o + n],
                        start=False, stop=True,
                    )
                h16 = wp.tile([P, HW], F16, tag="h16")
                nc.scalar.activation(
                    h16[:, 0:cap], h_ps[:, 0:cap], RELU, bias=b1c
                )
                # ---- za[e] = h_blk.T @ w2; a = sigmoid(za + b2) ----
                for j in range(nb):
                    nc.tensor.matmul(
                        mtp[:, 320 + j : 321 + j],
                        h16[:, j * P : (j + 1) * P], w2h_s,
                        start=True, stop=True, skip_group_check=True,
                    )
                nc.scalar.activation(
                    a_s[:, k * 5 : k * 5 + nb],
                    mtp[:, 320 : 320 + nb], SIGMOID, bias=b2c,
                )
                # ---- phase B: ohka = ohk * a (one broadcast mult) ----
                ohkaB = wp.tile([P, HW], F16, tag="ohka")
                a_bc = a_s[:, k * 5 : k * 5 + nb, None].broadcast_to(
                    (P, nb, P)
                )
                nc.vector.tensor_tensor(
                    out=ohkaB[:, 0:cap], in0=ohk_s[:, e0 : e0 + cap],
                    in1=a_bc, op=MULT,
                )
                for j in range(nb):
                    sl = slice(e0 + j * P, e0 + (j + 1) * P)
                    nc.tensor.matmul(
                        mtp[:, 0:P], ohlo_s[:, sl],
                        ohkaB[:, j * P : (j + 1) * P],
                        start=(j == 0), stop=(j == nb - 1),
                        skip_group_check=True,
                    )
                m16T = wp.tile([P, P], F16, tag="m16T")
                nc.vector.tensor_copy(m16T[:], mtp[:, 0:P])
                mN_ps = mtp[:, 2 * P : 2 * P + P // 2].bitcast(F16)
                nc.tensor.matmul(
                    mN_ps, m16T[:], ident16[:], is_transpose=True,
                    start=True, stop=True, skip_group_check=True,
                )
                m16N = wp.tile([P, P], F16, tag="m16N")
                nc.vector.tensor_copy(m16N[:], mN_ps)
                # ---- tails ----
                nc.tensor.matmul(
                    acc[:], ttT8[:, k, :], m16T[:],
                    start=(k == 0), stop=(k == NW - 1), skip_group_check=True,
                )
                nc.tensor.matmul(
                    mtp[:, P : 2 * P], iwT_s, m16N[:],
                    start=True, stop=True, skip_group_check=True,
                )
                nc.vector.tensor_copy(
                    part_all[:, k * P : (k + 1) * P], mtp[:, P : 2 * P]
                )
                if k == NW // 2 - 1:
                    nc.sync.dma_start(
                        out_part[:, 0 : DIM // 2], part_all[:, 0 : DIM // 2]
                    )
                elif k == NW - 1:
                    nc.sync.dma_start(
                        out_part[:, DIM // 2 : DIM], part_all[:, DIM // 2 : DIM]
                    )

            out_sb = wp.tile([P, P], F32, tag="out_sb")
            nc.scalar.copy(out_sb[:], acc[:])
            nc.sync.dma_start(out_img[:], out_sb[:])

    nc.compile()
    return nc


_PROGRAMS = {}


def _get_program(blocks):
    key = tuple(blocks)
    if key not in _PROGRAMS:
        _PROGRAMS[key] = _build_program(list(blocks))
    return _PROGRAMS[key]


def _core_arrays(kloc, arb, order, blocks):
    """kloc: src-base (0..127) for this core's edges; arb: tgt values.
    order[k] = actual bucket handled by program slot k. Returns ohkt,
    ohlt (key-major), ohk, ohlo (edge-major) [P, ec] f8."""
    import ml_dtypes

    caps = [b * P for b in blocks]
    off = np.concatenate([[0], np.cumsum(caps)]).astype(int)
    ec = int(off[-1])
    w = arb >> 7
    lo = arb & 127
    klocs = np.full(ec, -1, np.int64)
    los = np.full(ec, -1, np.int64)
    slot_of = np.empty(NW, np.int64)
    slot_of[order] = np.arange(NW)
    fill = np.zeros(NW, np.int64)
    for i in range(len(kloc)):
        k = slot_of[w[i]]
        s = off[k] + fill[k]
        klocs[s] = kloc[i]
        los[s] = lo[i]
        fill[k] += 1
        assert fill[k] <= caps[k]
    f8 = ml_dtypes.float8_e4m3
    rng = np.arange(P)
    ohkt = np.ascontiguousarray((klocs[None, :] == rng[:, None]).astype(f8))
    ohlt = np.ascontiguousarray((los[None, :] == rng[:, None]).astype(f8))
    ohlo = np.zeros((P, ec), f8)
    ohk = np.zeros((P, ec), f8)
    nblk = ec // P
    losb = los.reshape(nblk, P)
    klocsb = klocs.reshape(nblk, P)
    for b in range(nblk):
        ohlo[:, b * P : (b + 1) * P] = (losb[b][:, None] == rng[None, :]).astype(f8)
        ohk[:, b * P : (b + 1) * P] = (klocsb[b][:, None] == rng[None, :]).astype(f8)
    return ohkt, ohlt, np.ascontiguousarray(ohk), np.ascontiguousarray(ohlo)


def _plan(src, tgt):
    """Per-core bucket order (fill desc) + shared capacity profile."""
    fills = np.zeros((NCORES, NW), np.int64)
    for c in range(NCORES):
        sel = (src >> 7) == c
        fills[c] = np.bincount(tgt[sel] >> 7, minlength=NW)
    orders = [np.argsort(-fills[c], kind="stable") for c in range(NCORES)]
    sorted_fills = -np.sort(-fills, axis=1)
    prof = sorted_fills.max(axis=0)
    blocks = [int(x) for x in np.ceil(prof / P).astype(int)]
    blocks = [min(max(b, 1), 5) for b in blocks]
    return orders, blocks


def _make_in_maps(img_features, text_features, src, tgt, W1, b1, w2, b2,
                  orders, blocks):
    img = np.asarray(img_features, dtype=np.float32)
    txt = np.asarray(text_features, dtype=np.float32)
    src = np.asarray(src).astype(np.int64)
    tgt = np.asarray(tgt).astype(np.int64)
    txt16f = txt.astype(np.float16)
    txtT = txt.T.astype(np.float16)                     # [1024, 128]
    w1i16 = W1[:, :P].T.astype(np.float16)
    w1x16 = W1[:, P:].T.astype(np.float16)
    cst = np.ascontiguousarray(
        np.stack(
            [np.asarray(b1, np.float32),
             np.full(P, np.float32(b2), np.float32)], axis=1)
    )
    cst16 = cst.view(np.float16)                        # [P, 4] f32 bits
    w2h = np.asarray(w2, np.float16).reshape(P, 1)
    pad = np.zeros((P, 1), np.float16)

    in_maps = []
    for c in range(NCORES):
        base = c * P
        order = orders[c]
        sel = (src >= base) & (src < base + P)
        ohkt, ohlt, ohk, ohlo = _core_arrays(
            src[sel] - base, tgt[sel], order, blocks
        )
        iw = img[:, base : base + P].astype(np.float16)
        blob = np.ascontiguousarray(np.concatenate(
            [w1i16, w1x16, iw, np.ascontiguousarray(iw.T), w2h, pad, cst16],
            axis=1))
        # permute text chunks so program slot k sees bucket order[k]
        t16 = np.empty((P, DIM), np.float16)
        tt8 = np.empty((P, NW, P), np.float16)
        for k in range(NW):
            wv = order[k]
            t16[:, k * P : (k + 1) * P] = txt16f[:, wv * P : (wv + 1) * P]
            tt8[:, k, :] = txtT[wv * P : (wv + 1) * P, :]
        m = {
            "txt16": np.ascontiguousarray(t16),
            "ttT8": np.ascontiguousarray(tt8.reshape(P, NW * P)),
            "blob16": blob,
            "ohkt": ohkt, "ohlt": ohlt, "ohlo": ohlo, "ohk": ohk,
        }
        in_maps.append(m)
    return in_maps


def _run(inputs, trace=False):
    from concourse.bass_utils import run_bass_kernel_spmd

    src = np.asarray(inputs["src"]).astype(np.int64)
    tgt = np.asarray(inputs["tgt"]).astype(np.int64)
    orders, blocks = _plan(src, tgt)
    nc = _get_program(blocks)
    in_maps = _make_in_maps(**inputs, orders=orders, blocks=blocks)
    res = run_bass_kernel_spmd(
        nc, in_maps, core_ids=list(range(NCORES)), trace=trace
    )
    att_img = np.concatenate([r["out_img"] for r in res.results], axis=1)
    att_txt = np.zeros((P, DIM), np.float32)
    for c in range(NCORES):
        part = res.results[c]["out_part"]
        order = orders[c]
        for k in range(NW):
            wv = order[k]
            att_txt[:, wv * P : (wv + 1) * P] += part[:, k * P : (k + 1) * P]
    return (
        np.ascontiguousarray(att_img.astype(np.float32)),
        np.ascontiguousarray(att_txt),
    ), res


def kernel(**inputs):
    out, _ = _run(inputs, trace=False)
    return out
